# revision 13
# baseline (speedup 1.0000x reference)
"""MultiHeadAttention Trainium2 kernel, 8-core SPMD.

Sharding: core = (batch b, head-group g), b in {0,1}, g in {0..3}.
Each core computes 4 heads of one batch (tensor-parallel on heads,
data-parallel on batch). Out-projection partials (and the bias) are
summed on host.

Structure per core:
  Phase 1a: Q+V projections (PE-dense, 8 PSUM banks).
  Interleaved region: K projection (2 PSUM banks, 256-col halves),
    attention per 512-chunk (scores pairs in 2-bank PSUM mega-tiles ->
    one exp -> PV; causal mask folded into PE as a -30000 bias matmul;
    softmax row-sums accumulate on the Pool engine with one
    ones-matmul per (chunk, head)), then out-projection (reusing the
    K-proj PSUM banks). The tile scheduler interleaves these streams so
    the PE always has independent GEMM work while Act/Pool/DVE chase
    the attention chain.

All matmul inputs fp16 (full PE rate at any tile size); accumulation
fp32 in PSUM. Self-contained: hardcodes B=2, S=2048, D=2048, H=16.
"""

import numpy as np

import concourse.bacc as bacc
import concourse.mybir as mybir
import concourse.tile as tile
from concourse.bass_utils import run_bass_kernel_spmd

B, S, D = 2, 2048, 2048
H = 16
HD = D // H          # 128 head dim
G = 4                # head groups (tensor parallel degree)
HPG = H // G         # 4 heads per group
DG = HPG * HD        # 512 features per group
NCORES = 8
NTC = D // 128       # 16 contraction chunks
NIT = S // 128       # 16 seq tiles of 128
NSC = S // 512       # 4 seq chunks of 512
SCALE = float(1.0 / np.sqrt(np.float32(S)))

F32 = mybir.dt.float32
F32R = mybir.dt.float32r
F16 = mybir.dt.float16
EXP = mybir.ActivationFunctionType.Exp

_CACHE = {}


def _build(nreps=1, trace_sim=False):
    nc = bacc.Bacc(target_bir_lowering=False, trn_type="TRN2")
    xT = nc.dram_tensor("xT", [D, S], F16, kind="ExternalInput")
    wqT = nc.dram_tensor("wqT", [D, DG], F16, kind="ExternalInput")
    wkT = nc.dram_tensor("wkT", [D, DG], F16, kind="ExternalInput")
    wvT = nc.dram_tensor("wvT", [D, DG], F16, kind="ExternalInput")
    woT = nc.dram_tensor("woT", [DG, D], F16, kind="ExternalInput")
    maskm = nc.dram_tensor("maskm", [128, 128], F16, kind="ExternalInput")
    ones = nc.dram_tensor("ones", [128, 128], F32R, kind="ExternalInput")
    y = nc.dram_tensor("y", [S, D], F16, kind="ExternalOutput")

    with tile.TileContext(nc, trace_sim=trace_sim) as tc:
      for _rep in range(nreps):
        with tc.tile_pool(name="res", bufs=1) as res:
            qt = [res.tile([128, S], F16, tag=f"qt{h}", name=f"qt{h}") for h in range(HPG)]
            kt = [res.tile([128, S], F16, tag=f"kt{h}", name=f"kt{h}") for h in range(HPG)]
            vg = [res.tile([128, 4 * DG], F16, tag=f"vg{j}", name=f"vg{j}") for j in range(4)]
            ctxt = [res.tile([128, S], F16, tag=f"ctx{h}", name=f"ctx{h}") for h in range(HPG)]
            wo = [res.tile([128, D], F16, tag=f"wo{h}", name=f"wo{h}") for h in range(HPG)]
            wkg = [res.tile([128, 4 * DG], F16, tag=f"wkg{g}", name=f"wkg{g}") for g in range(4)]
            maskm_t = res.tile([128, 128], F16, tag="maskm", name="maskm_t")
            ones_t = res.tile([128, 128], F32R, tag="ones", name="ones_t")


            with (
                tc.tile_pool(name="wqv", bufs=1) as wp,
                tc.tile_pool(name="xstream", bufs=8) as xp,
                tc.tile_pool(name="ph2w", bufs=3) as etp,
                tc.tile_pool(name="rsa", bufs=2) as rsp,
            ):
                wqg = [wp.tile([128, 4 * DG], F16, tag=f"wqg{g}", name=f"wqg{g}") for g in range(4)]
                wvg = [wp.tile([128, 4 * DG], F16, tag=f"wvg{g}", name=f"wvg{g}") for g in range(4)]
                # weights: scalar queue, first-needed first
                for g4 in range(4):
                    nc.scalar.dma_start(
                        wqg[g4][:].rearrange("p (g d) -> p g d", g=4),
                        wqT[g4 * 512 : (g4 + 1) * 512, :].rearrange("(g p) d -> p g d", p=128),
                    )
                    nc.scalar.dma_start(
                        wvg[g4][:].rearrange("p (g d) -> p g d", g=4),
                        wvT[g4 * 512 : (g4 + 1) * 512, :].rearrange("(g p) d -> p g d", p=128),
                    )
                for g4 in range(4):
                    nc.scalar.dma_start(
                        wkg[g4][:].rearrange("p (g d) -> p g d", g=4),
                        wkT[g4 * 512 : (g4 + 1) * 512, :].rearrange("(g p) d -> p g d", p=128),
                    )
                nc.scalar.dma_start(maskm_t[:], maskm[:])
                nc.scalar.dma_start(ones_t[:], ones[:])
                for h in range(HPG):
                    nc.scalar.dma_start(wo[h][:], woT[h * 128 : (h + 1) * 128, :])

                # ---- Phase 1a: Q + V projections ----
                with tc.tile_pool(name="ps1", bufs=8, space="PSUM") as pp1:
                    for ic in range(NSC):
                        i0 = ic * 512
                        qps = [pp1.tile([128, 512], F32, tag="projps", name="projps") for _ in range(HPG)]
                        vps = [pp1.tile([128, DG], F32, tag="projps", name="projps") for _ in range(4)]
                        for g4 in range(4):
                            xtg = xp.tile([128, 4 * 512], F16, tag="xt", name="xt")
                            nc.sync.dma_start(
                                xtg[:].rearrange("p (g i) -> p g i", g=4),
                                xT[g4 * 512 : (g4 + 1) * 512, i0 : i0 + 512].rearrange(
                                    "(g p) i -> p g i", p=128
                                ),
                            )
                            for g in range(4):
                                c = g4 * 4 + g
                                st = c == 0
                                sp = c == NTC - 1
                                xt_c = xtg[:, g * 512 : (g + 1) * 512]
                                for h in range(HPG):
                                    nc.tensor.matmul(
                                        qps[h][:],
                                        wqg[g4][:, g * 512 + h * 128 : g * 512 + (h + 1) * 128],
                                        xt_c,
                                        start=st, stop=sp,
                                    )
                                for jj in range(4):
                                    nc.tensor.matmul(
                                        vps[jj][:],
                                        xtg[:, g * 512 + jj * 128 : g * 512 + (jj + 1) * 128],
                                        wvg[g4][:, g * 512 : (g + 1) * 512],
                                        start=st, stop=sp,
                                    )
                        for h in range(HPG):
                            nc.vector.tensor_copy(qt[h][:, i0 : i0 + 512], qps[h][:])
                        for jj in range(4):
                            nc.vector.tensor_copy(
                                vg[ic][:, jj * DG : (jj + 1) * DG], vps[jj][:]
                            )

                # ---- Interleaved region: K proj + attention + out-proj ----
                with tc.tile_pool(name="psI", bufs=2, space="PSUM") as psp:
                    for ic in range(NSC):
                        i0 = ic * 512
                        # K projection for this chunk: 2 PSUM banks, two
                        # 256-col halves; 4 xtg tiles stay live across halves.
                        xtgs = []
                        for g4 in range(4):
                            xtg = xp.tile([128, 4 * 512], F16, tag="xt", name="xt")
                            nc.sync.dma_start(
                                xtg[:].rearrange("p (g i) -> p g i", g=4),
                                xT[g4 * 512 : (g4 + 1) * 512, i0 : i0 + 512].rearrange(
                                    "(g p) i -> p g i", p=128
                                ),
                            )
                            xtgs.append(xtg)
                        for hpair in range(2):
                            km = [psp.tile([128, 512], F32, tag="kps", name="kps", bufs=2) for _ in range(2)]
                            for g4 in range(4):
                                for g in range(4):
                                    c = g4 * 4 + g
                                    st = c == 0
                                    sp = c == NTC - 1
                                    xt_c = xtgs[g4][:, g * 512 : (g + 1) * 512]
                                    for hh in range(2):
                                        h = 2 * hpair + hh
                                        nc.tensor.matmul(
                                            km[hh][:],
                                            wkg[g4][:, g * 512 + h * 128 : g * 512 + (h + 1) * 128],
                                            xt_c,
                                            start=st, stop=sp,
                                        )
                            for hh in range(2):
                                h = 2 * hpair + hh
                                nc.vector.tensor_copy(
                                    kt[h][:, i0 : i0 + 512], km[hh][:]
                                )

                        # Attention for this chunk.
                        nj = 4 * (ic + 1)
                        for h in range(HPG):
                            ctxps = psp.tile([128, 512], F32, tag="ctxps", name="ctxps", bufs=2)
                            rsacc = rsp.tile([128, 512], F32R, tag="rsacc", name="rsacc", bufs=3)
                            for p in range(nj // 2):
                                stp = psp.tile([128, 1024], F32, tag="stps", name="stps", bufs=2)
                                et = etp.tile([128, 1024], F16, tag="et", name="et", bufs=6)
                                rels = []
                                for k2 in (0, 1):
                                    jb = 2 * p + k2
                                    j0 = jb * 128
                                    rel = max(i0, j0) - i0
                                    base = 512 * k2
                                    nc.tensor.matmul(
                                        stp[:, base + rel : base + 512],
                                        kt[h][:, j0 : j0 + 128],
                                        qt[h][:, i0 + rel : i0 + 512],
                                        start=True, stop=True,
                                    )
                                    rels.append(rel)
                                # One exp over both tiles; the gap region of
                                # diagonal pairs holds stale-PSUM exp values,
                                # never read downstream.
                                nc.scalar.activation(
                                    et[:, rels[0] : 1024], stp[:, rels[0] : 1024],
                                    EXP, bias=0.0, scale=SCALE,
                                )
                                for k2 in (0, 1):
                                    jb = 2 * p + k2
                                    rel = rels[k2]
                                    base = 512 * k2
                                    if 2 * p + k2 >= 4 * ic:
                                        nc.vector.tensor_mul(
                                            et[:, base + rel : base + rel + 128],
                                            et[:, base + rel : base + rel + 128],
                                            maskm_t[:],
                                        )
                                    if jb == 0:
                                        nc.gpsimd.tensor_copy(rsacc[:], et[:, 0:512])
                                    else:
                                        nc.gpsimd.tensor_add(
                                            rsacc[:, rel:512],
                                            rsacc[:, rel:512],
                                            et[:, base + rel : base + 512],
                                        )
                                    nc.tensor.matmul(
                                        ctxps[:, rel:512],
                                        vg[jb // 4][
                                            :, (jb % 4) * DG + h * 128 : (jb % 4) * DG + (h + 1) * 128
                                        ],
                                        et[:, base + rel : base + 512],
                                        start=(jb == 0), stop=(jb == nj - 1),
                                    )
                            rsps = psp.tile([128, 512], F32, tag="stps", name="rsps", bufs=2)
                            nc.tensor.matmul(
                                rsps[:], ones_t[:], rsacc[:], start=True, stop=True
                            )
                            rrb = etp.tile([128, 512], F32, tag="rrb", name="rrb")
                            nc.vector.reciprocal(rrb[:], rsps[:])
                            nc.vector.tensor_mul(
                                ctxt[h][:, i0 : i0 + 512], ctxps[:], rrb[:]
                            )

                    # ---- Out-projection (reuses the K-proj PSUM banks) ----
                    for it in range(NIT):
                        t0 = it * 128
                        ysb = etp.tile([128, D], F16, tag="ysb", name="ysb")
                        for oc in range(4):
                            o0 = oc * 512
                            yps = psp.tile([128, 512], F32, tag="kps", name="yps", bufs=2)
                            for h in range(HPG):
                                nc.tensor.matmul(
                                    yps[:],
                                    ctxt[h][:, t0 : t0 + 128],
                                    wo[h][:, o0 : o0 + 512],
                                    start=(h == 0), stop=(h == HPG - 1),
                                )
                            nc.vector.tensor_copy(ysb[:, o0 : o0 + 512], yps[:])
                        nc.sync.dma_start(y[t0 : t0 + 128, :], ysb[:])
    nc.finalize()
    return nc


def get_nc():
    if "nc" not in _CACHE:
        _CACHE["nc"] = _build()
    return _CACHE["nc"]


def make_in_maps(inputs, w_q, w_k, w_v, w_o, b_o):
    x = np.asarray(inputs, dtype=np.float32)
    w_q = np.asarray(w_q, dtype=np.float32)
    w_k = np.asarray(w_k, dtype=np.float32)
    w_v = np.asarray(w_v, dtype=np.float32)
    w_o = np.asarray(w_o, dtype=np.float32)

    ones = np.ones((128, 128), dtype=np.float32)

    xTs = [np.ascontiguousarray(x[b].T.astype(np.float16)) for b in range(B)]
    wqTs = [np.ascontiguousarray(w_q[g * DG : (g + 1) * DG, :].T.astype(np.float16)) for g in range(G)]
    wkTs = [np.ascontiguousarray(w_k[g * DG : (g + 1) * DG, :].T.astype(np.float16)) for g in range(G)]
    wvTs = [np.ascontiguousarray(w_v[g * DG : (g + 1) * DG, :].T.astype(np.float16)) for g in range(G)]
    woTs = [np.ascontiguousarray(w_o[:, g * DG : (g + 1) * DG].T.astype(np.float16)) for g in range(G)]

    in_maps = []
    for core in range(NCORES):
        b, g = divmod(core, G)
        in_maps.append(
            {
                "xT": xTs[b],
                "wqT": wqTs[g],
                "wkT": wkTs[g],
                "wvT": wvTs[g],
                "woT": woTs[g],
                "maskm": np.triu(np.ones((128, 128), dtype=np.float16)),
                "ones": ones,
            }
        )
    return in_maps


def assemble(results, b_o):
    out = np.zeros((B, S, D), dtype=np.float32)
    for core in range(NCORES):
        b = core // G
        out[b] += results[core]["y"].astype(np.float32)
    out += np.asarray(b_o, dtype=np.float32)[None, None, :]
    return out


def kernel(inputs, w_q, w_k, w_v, w_o, b_o):
    nc = get_nc()
    in_maps = make_in_maps(inputs, w_q, w_k, w_v, w_o, b_o)
    res = run_bass_kernel_spmd(nc, in_maps, core_ids=list(range(NCORES)))
    return assemble(res.results, b_o)


# revision 14
# speedup vs baseline: 1.0872x; 1.0872x over previous
"""MultiHeadAttention Trainium2 kernel, 8-core SPMD.

Sharding: core = (batch b, head-group g), b in {0,1}, g in {0..3}.
Each core computes 4 heads of one batch (tensor-parallel on heads,
data-parallel on batch). Out-projection partials (and the bias) are
summed on host.

Structure per core:
  Phase 1a: Q+V projections (PE-dense, 8 PSUM banks).
  Interleaved region: K projection (2 PSUM banks, 256-col halves),
    attention per 512-chunk (scores pairs in 2-bank PSUM mega-tiles ->
    one exp -> PV; causal mask folded into PE as a -30000 bias matmul;
    softmax row-sums accumulate on the Pool engine with one
    ones-matmul per (chunk, head)), then out-projection (reusing the
    K-proj PSUM banks). The tile scheduler interleaves these streams so
    the PE always has independent GEMM work while Act/Pool/DVE chase
    the attention chain.

All matmul inputs fp16 (full PE rate at any tile size); accumulation
fp32 in PSUM. Self-contained: hardcodes B=2, S=2048, D=2048, H=16.
"""

import numpy as np

import concourse.bacc as bacc
import concourse.mybir as mybir
import concourse.tile as tile
from concourse.bass_utils import run_bass_kernel_spmd

B, S, D = 2, 2048, 2048
H = 16
HD = D // H          # 128 head dim
G = 4                # head groups (tensor parallel degree)
HPG = H // G         # 4 heads per group
DG = HPG * HD        # 512 features per group
NCORES = 8
NTC = D // 128       # 16 contraction chunks
NIT = S // 128       # 16 seq tiles of 128
NSC = S // 512       # 4 seq chunks of 512
SCALE = float(1.0 / np.sqrt(np.float32(S)))

F32 = mybir.dt.float32
F32R = mybir.dt.float32r
F16 = mybir.dt.float16
EXP = mybir.ActivationFunctionType.Exp

_CACHE = {}


def _build(nreps=1, trace_sim=False):
    nc = bacc.Bacc(target_bir_lowering=False, trn_type="TRN2")
    xT = nc.dram_tensor("xT", [D, S], F16, kind="ExternalInput")
    wqT = nc.dram_tensor("wqT", [D, DG], F16, kind="ExternalInput")
    wkT = nc.dram_tensor("wkT", [D, DG], F16, kind="ExternalInput")
    wvT = nc.dram_tensor("wvT", [D, DG], F16, kind="ExternalInput")
    woT = nc.dram_tensor("woT", [DG, D], F16, kind="ExternalInput")
    maskm = nc.dram_tensor("maskm", [128, 128], F16, kind="ExternalInput")
    ones = nc.dram_tensor("ones", [128, 128], F32R, kind="ExternalInput")
    y = nc.dram_tensor("y", [S, D], F16, kind="ExternalOutput")

    with tile.TileContext(nc, trace_sim=trace_sim) as tc:
      for _rep in range(nreps):
        with tc.tile_pool(name="res", bufs=1) as res:
            qt = [res.tile([128, S], F16, tag=f"qt{h}", name=f"qt{h}") for h in range(HPG)]
            kt = [res.tile([128, S], F16, tag=f"kt{h}", name=f"kt{h}") for h in range(HPG)]
            vg = [res.tile([128, 4 * DG], F16, tag=f"vg{j}", name=f"vg{j}") for j in range(4)]
            ctxt = [res.tile([128, S], F16, tag=f"ctx{h}", name=f"ctx{h}") for h in range(HPG)]
            wo = [res.tile([128, D], F16, tag=f"wo{h}", name=f"wo{h}") for h in range(HPG)]
            wkg = [res.tile([128, 4 * DG], F16, tag=f"wkg{g}", name=f"wkg{g}") for g in range(4)]
            maskm_t = res.tile([128, 128], F16, tag="maskm", name="maskm_t")
            ones_t = res.tile([128, 128], F32R, tag="ones", name="ones_t")


            with (
                tc.tile_pool(name="wqv", bufs=1) as wp,
                tc.tile_pool(name="xstream", bufs=8) as xp,
                tc.tile_pool(name="ph2w", bufs=3) as etp,
                tc.tile_pool(name="rsa", bufs=2) as rsp,
            ):
                wqg = [wp.tile([128, 4 * DG], F16, tag=f"wqg{g}", name=f"wqg{g}") for g in range(4)]
                wvg = [wp.tile([128, 4 * DG], F16, tag=f"wvg{g}", name=f"wvg{g}") for g in range(4)]
                # weights: scalar queue, first-needed first
                for g4 in range(4):
                    nc.scalar.dma_start(
                        wqg[g4][:].rearrange("p (g d) -> p g d", g=4),
                        wqT[g4 * 512 : (g4 + 1) * 512, :].rearrange("(g p) d -> p g d", p=128),
                    )
                    nc.scalar.dma_start(
                        wvg[g4][:].rearrange("p (g d) -> p g d", g=4),
                        wvT[g4 * 512 : (g4 + 1) * 512, :].rearrange("(g p) d -> p g d", p=128),
                    )
                for g4 in range(4):
                    nc.scalar.dma_start(
                        wkg[g4][:].rearrange("p (g d) -> p g d", g=4),
                        wkT[g4 * 512 : (g4 + 1) * 512, :].rearrange("(g p) d -> p g d", p=128),
                    )
                nc.scalar.dma_start(maskm_t[:], maskm[:])
                nc.scalar.dma_start(ones_t[:], ones[:])
                for h in range(HPG):
                    nc.scalar.dma_start(wo[h][:], woT[h * 128 : (h + 1) * 128, :])

                # ---- Phase 1a: Q + V projections ----
                with tc.tile_pool(name="ps1", bufs=8, space="PSUM") as pp1:
                    for ic in range(NSC):
                        i0 = ic * 512
                        qps = [pp1.tile([128, 512], F32, tag="projps", name="projps") for _ in range(HPG)]
                        vps = [pp1.tile([128, DG], F32, tag="projps", name="projps") for _ in range(4)]
                        for g4 in range(4):
                            xtg = xp.tile([128, 4 * 512], F16, tag="xt", name="xt")
                            nc.sync.dma_start(
                                xtg[:].rearrange("p (g i) -> p g i", g=4),
                                xT[g4 * 512 : (g4 + 1) * 512, i0 : i0 + 512].rearrange(
                                    "(g p) i -> p g i", p=128
                                ),
                            )
                            for g in range(4):
                                c = g4 * 4 + g
                                st = c == 0
                                sp = c == NTC - 1
                                xt_c = xtg[:, g * 512 : (g + 1) * 512]
                                for h in range(HPG):
                                    nc.tensor.matmul(
                                        qps[h][:],
                                        wqg[g4][:, g * 512 + h * 128 : g * 512 + (h + 1) * 128],
                                        xt_c,
                                        start=st, stop=sp,
                                    )
                                for jj in range(4):
                                    nc.tensor.matmul(
                                        vps[jj][:],
                                        xtg[:, g * 512 + jj * 128 : g * 512 + (jj + 1) * 128],
                                        wvg[g4][:, g * 512 : (g + 1) * 512],
                                        start=st, stop=sp,
                                    )
                        for h in range(HPG):
                            nc.vector.tensor_copy(qt[h][:, i0 : i0 + 512], qps[h][:])
                        for jj in range(4):
                            nc.vector.tensor_copy(
                                vg[ic][:, jj * DG : (jj + 1) * DG], vps[jj][:]
                            )

                # ---- Interleaved region: K proj + attention + out-proj ----
                with tc.tile_pool(name="psI", bufs=2, space="PSUM") as psp:
                    for ic in range(NSC):
                        i0 = ic * 512
                        # K projection for this chunk: 2 PSUM banks, two
                        # 256-col halves; 4 xtg tiles stay live across halves.
                        xtgs = []
                        for g4 in range(4):
                            xtg = xp.tile([128, 4 * 512], F16, tag="xt", name="xt")
                            nc.sync.dma_start(
                                xtg[:].rearrange("p (g i) -> p g i", g=4),
                                xT[g4 * 512 : (g4 + 1) * 512, i0 : i0 + 512].rearrange(
                                    "(g p) i -> p g i", p=128
                                ),
                            )
                            xtgs.append(xtg)
                        for hpair in range(2):
                            km = [psp.tile([128, 512], F32, tag="kps", name="kps", bufs=2) for _ in range(2)]
                            for g4 in range(4):
                                for g in range(4):
                                    c = g4 * 4 + g
                                    st = c == 0
                                    sp = c == NTC - 1
                                    xt_c = xtgs[g4][:, g * 512 : (g + 1) * 512]
                                    for hh in range(2):
                                        h = 2 * hpair + hh
                                        nc.tensor.matmul(
                                            km[hh][:],
                                            wkg[g4][:, g * 512 + h * 128 : g * 512 + (h + 1) * 128],
                                            xt_c,
                                            start=st, stop=sp,
                                        )
                            for hh in range(2):
                                h = 2 * hpair + hh
                                nc.vector.tensor_copy(
                                    kt[h][:, i0 : i0 + 512], km[hh][:]
                                )

                        # Attention for this chunk.
                        nj = 4 * (ic + 1)
                        for h in range(HPG):
                            ctxps = psp.tile([128, 512], F32, tag="ctxps", name="ctxps", bufs=1)
                            rsacc = rsp.tile([128, 512], F32R, tag="rsacc", name="rsacc", bufs=3)
                            for p in range(nj // 2):
                                stp = psp.tile([128, 1024], F32, tag="stps", name="stps", bufs=2)
                                et = etp.tile([128, 1024], F16, tag="et", name="et", bufs=6)
                                rels = []
                                for k2 in (0, 1):
                                    jb = 2 * p + k2
                                    j0 = jb * 128
                                    rel = max(i0, j0) - i0
                                    base = 512 * k2
                                    nc.tensor.matmul(
                                        stp[:, base + rel : base + 512],
                                        kt[h][:, j0 : j0 + 128],
                                        qt[h][:, i0 + rel : i0 + 512],
                                        start=True, stop=True,
                                    )
                                    rels.append(rel)
                                # One exp over both tiles; the gap region of
                                # diagonal pairs holds stale-PSUM exp values,
                                # never read downstream.
                                nc.scalar.activation(
                                    et[:, rels[0] : 1024], stp[:, rels[0] : 1024],
                                    EXP, bias=0.0, scale=SCALE,
                                )
                                for k2 in (0, 1):
                                    jb = 2 * p + k2
                                    rel = rels[k2]
                                    base = 512 * k2
                                    if 2 * p + k2 >= 4 * ic:
                                        nc.vector.tensor_mul(
                                            et[:, base + rel : base + rel + 128],
                                            et[:, base + rel : base + rel + 128],
                                            maskm_t[:],
                                        )
                                    if jb == 0:
                                        nc.gpsimd.tensor_copy(rsacc[:], et[:, 0:512])
                                    else:
                                        nc.gpsimd.tensor_add(
                                            rsacc[:, rel:512],
                                            rsacc[:, rel:512],
                                            et[:, base + rel : base + 512],
                                        )
                                    nc.tensor.matmul(
                                        ctxps[:, rel:512],
                                        vg[jb // 4][
                                            :, (jb % 4) * DG + h * 128 : (jb % 4) * DG + (h + 1) * 128
                                        ],
                                        et[:, base + rel : base + 512],
                                        start=(jb == 0), stop=(jb == nj - 1),
                                    )
                            rsps = psp.tile([128, 512], F32, tag="rsps", name="rsps", bufs=1)
                            nc.tensor.matmul(
                                rsps[:], ones_t[:], rsacc[:], start=True, stop=True
                            )
                            rrb = etp.tile([128, 512], F32, tag="rrb", name="rrb")
                            nc.vector.reciprocal(rrb[:], rsps[:])
                            nc.vector.tensor_mul(
                                ctxt[h][:, i0 : i0 + 512], ctxps[:], rrb[:]
                            )

                    # ---- Out-projection (reuses the K-proj PSUM banks) ----
                    for it in range(NIT):
                        t0 = it * 128
                        ysb = etp.tile([128, D], F16, tag="ysb", name="ysb")
                        for oc in range(4):
                            o0 = oc * 512
                            yps = psp.tile([128, 512], F32, tag="kps", name="yps", bufs=2)
                            for h in range(HPG):
                                nc.tensor.matmul(
                                    yps[:],
                                    ctxt[h][:, t0 : t0 + 128],
                                    wo[h][:, o0 : o0 + 512],
                                    start=(h == 0), stop=(h == HPG - 1),
                                )
                            nc.vector.tensor_copy(ysb[:, o0 : o0 + 512], yps[:])
                        nc.sync.dma_start(y[t0 : t0 + 128, :], ysb[:])
    nc.finalize()
    return nc


def get_nc():
    if "nc" not in _CACHE:
        _CACHE["nc"] = _build()
    return _CACHE["nc"]


def make_in_maps(inputs, w_q, w_k, w_v, w_o, b_o):
    x = np.asarray(inputs, dtype=np.float32)
    w_q = np.asarray(w_q, dtype=np.float32)
    w_k = np.asarray(w_k, dtype=np.float32)
    w_v = np.asarray(w_v, dtype=np.float32)
    w_o = np.asarray(w_o, dtype=np.float32)

    ones = np.ones((128, 128), dtype=np.float32)

    xTs = [np.ascontiguousarray(x[b].T.astype(np.float16)) for b in range(B)]
    wqTs = [np.ascontiguousarray(w_q[g * DG : (g + 1) * DG, :].T.astype(np.float16)) for g in range(G)]
    wkTs = [np.ascontiguousarray(w_k[g * DG : (g + 1) * DG, :].T.astype(np.float16)) for g in range(G)]
    wvTs = [np.ascontiguousarray(w_v[g * DG : (g + 1) * DG, :].T.astype(np.float16)) for g in range(G)]
    woTs = [np.ascontiguousarray(w_o[:, g * DG : (g + 1) * DG].T.astype(np.float16)) for g in range(G)]

    in_maps = []
    for core in range(NCORES):
        b, g = divmod(core, G)
        in_maps.append(
            {
                "xT": xTs[b],
                "wqT": wqTs[g],
                "wkT": wkTs[g],
                "wvT": wvTs[g],
                "woT": woTs[g],
                "maskm": np.triu(np.ones((128, 128), dtype=np.float16)),
                "ones": ones,
            }
        )
    return in_maps


def assemble(results, b_o):
    out = np.zeros((B, S, D), dtype=np.float32)
    for core in range(NCORES):
        b = core // G
        out[b] += results[core]["y"].astype(np.float32)
    out += np.asarray(b_o, dtype=np.float32)[None, None, :]
    return out


def kernel(inputs, w_q, w_k, w_v, w_o, b_o):
    nc = get_nc()
    in_maps = make_in_maps(inputs, w_q, w_k, w_v, w_o, b_o)
    res = run_bass_kernel_spmd(nc, in_maps, core_ids=list(range(NCORES)))
    return assemble(res.results, b_o)


# revision 16
# speedup vs baseline: 1.1637x; 1.0704x over previous
"""MultiHeadAttention Trainium2 kernel, 8-core SPMD.

Sharding: core = (batch b, head-group g), b in {0,1}, g in {0..3}.
Each core computes 4 heads of one batch (tensor-parallel on heads,
data-parallel on batch). Out-projection partials (and the bias) are
summed on host.

Structure per core:
  Phase 1a: Q+V projections (PE-dense, 8 PSUM banks).
  Interleaved region: K and Q projections (2 shared PSUM banks, one full-width
    accumulation stream per bank, heads in pairs x two contraction
    passes), attention per 512-chunk (scores pairs in 2-bank PSUM
    mega-tiles -> one exp -> DVE mask on diagonal squares -> PV;
    softmax row-sums accumulate on the Pool engine with one
    ones-matmul per (chunk, head)), then out-projection (reusing the
    K-proj PSUM banks). The tile scheduler interleaves these streams so
    the PE always has independent GEMM work while Act/Pool/DVE chase
    the attention chain.

All matmul inputs fp16 (full PE rate at any tile size); accumulation
fp32 in PSUM. Self-contained: hardcodes B=2, S=2048, D=2048, H=16.
"""

import numpy as np

import concourse.bacc as bacc
import concourse.mybir as mybir
import concourse.tile as tile
from concourse.bass_utils import run_bass_kernel_spmd

B, S, D = 2, 2048, 2048
H = 16
HD = D // H          # 128 head dim
G = 4                # head groups (tensor parallel degree)
HPG = H // G         # 4 heads per group
DG = HPG * HD        # 512 features per group
NCORES = 8
NTC = D // 128       # 16 contraction chunks
NIT = S // 128       # 16 seq tiles of 128
NSC = S // 512       # 4 seq chunks of 512
SCALE = float(1.0 / np.sqrt(np.float32(S)))

F32 = mybir.dt.float32
F32R = mybir.dt.float32r
F16 = mybir.dt.float16
EXP = mybir.ActivationFunctionType.Exp

_CACHE = {}


def _build(nreps=1, trace_sim=False):
    nc = bacc.Bacc(target_bir_lowering=False, trn_type="TRN2")
    xT = nc.dram_tensor("xT", [D, S], F16, kind="ExternalInput")
    wqT = nc.dram_tensor("wqT", [D, DG], F16, kind="ExternalInput")
    wkT = nc.dram_tensor("wkT", [D, DG], F16, kind="ExternalInput")
    wvT = nc.dram_tensor("wvT", [D, DG], F16, kind="ExternalInput")
    woT = nc.dram_tensor("woT", [DG, D], F16, kind="ExternalInput")
    maskm = nc.dram_tensor("maskm", [128, 128], F16, kind="ExternalInput")
    ones = nc.dram_tensor("ones", [128, 128], F32R, kind="ExternalInput")
    y = nc.dram_tensor("y", [S, D], F16, kind="ExternalOutput")

    with tile.TileContext(nc, trace_sim=trace_sim) as tc:
      for _rep in range(nreps):
        with tc.tile_pool(name="res", bufs=1) as res:
            qt = [res.tile([128, S], F16, tag=f"qt{h}", name=f"qt{h}") for h in range(HPG)]
            kt = [res.tile([128, S], F16, tag=f"kt{h}", name=f"kt{h}") for h in range(HPG)]
            vg = [res.tile([128, 4 * DG], F16, tag=f"vg{j}", name=f"vg{j}") for j in range(4)]
            ctxt = [res.tile([128, S], F16, tag=f"ctx{h}", name=f"ctx{h}") for h in range(HPG)]
            wo = [res.tile([128, D], F16, tag=f"wo{h}", name=f"wo{h}") for h in range(HPG)]
            wkg = [res.tile([128, 4 * DG], F16, tag=f"wkg{g}", name=f"wkg{g}") for g in range(4)]
            maskm_t = res.tile([128, 128], F16, tag="maskm", name="maskm_t")
            ones_t = res.tile([128, 128], F32R, tag="ones", name="ones_t")


            with (
                tc.tile_pool(name="wqv", bufs=1) as wp,
                tc.tile_pool(name="xstream", bufs=8) as xp,
                tc.tile_pool(name="ph2w", bufs=3) as etp,
                tc.tile_pool(name="rsa", bufs=2) as rsp,
            ):
                wqg = [wp.tile([128, 4 * DG], F16, tag=f"wqg{g}", name=f"wqg{g}") for g in range(4)]
                wvg = [wp.tile([128, 4 * DG], F16, tag=f"wvg{g}", name=f"wvg{g}") for g in range(4)]
                # weights: scalar queue, first-needed first
                for g4 in range(4):
                    nc.scalar.dma_start(
                        wqg[g4][:].rearrange("p (g d) -> p g d", g=4),
                        wqT[g4 * 512 : (g4 + 1) * 512, :].rearrange("(g p) d -> p g d", p=128),
                    )
                    nc.scalar.dma_start(
                        wvg[g4][:].rearrange("p (g d) -> p g d", g=4),
                        wvT[g4 * 512 : (g4 + 1) * 512, :].rearrange("(g p) d -> p g d", p=128),
                    )
                for g4 in range(4):
                    nc.scalar.dma_start(
                        wkg[g4][:].rearrange("p (g d) -> p g d", g=4),
                        wkT[g4 * 512 : (g4 + 1) * 512, :].rearrange("(g p) d -> p g d", p=128),
                    )
                nc.scalar.dma_start(maskm_t[:], maskm[:])
                nc.scalar.dma_start(ones_t[:], ones[:])
                for h in range(HPG):
                    nc.scalar.dma_start(wo[h][:], woT[h * 128 : (h + 1) * 128, :])

                # ---- Phase 1a: V projection ----
                with tc.tile_pool(name="ps1", bufs=8, space="PSUM") as pp1:
                    for ic in range(NSC):
                        i0 = ic * 512
                        vps = [pp1.tile([128, DG], F32, tag="projps", name="projps") for _ in range(4)]
                        for g4 in range(4):
                            xtg = xp.tile([128, 4 * 512], F16, tag="xt", name="xt")
                            nc.sync.dma_start(
                                xtg[:].rearrange("p (g i) -> p g i", g=4),
                                xT[g4 * 512 : (g4 + 1) * 512, i0 : i0 + 512].rearrange(
                                    "(g p) i -> p g i", p=128
                                ),
                            )
                            for g in range(4):
                                c = g4 * 4 + g
                                st = c == 0
                                sp = c == NTC - 1
                                for jj in range(4):
                                    nc.tensor.matmul(
                                        vps[jj][:],
                                        xtg[:, g * 512 + jj * 128 : g * 512 + (jj + 1) * 128],
                                        wvg[g4][:, g * 512 : (g + 1) * 512],
                                        start=st, stop=sp,
                                    )
                        for jj in range(4):
                            nc.vector.tensor_copy(
                                vg[ic][:, jj * DG : (jj + 1) * DG], vps[jj][:]
                            )

                # ---- Interleaved region: K proj + attention + out-proj ----
                with tc.tile_pool(name="psI", bufs=2, space="PSUM") as psp:
                    for ic in range(NSC):
                        i0 = ic * 512
                        # K projection for this chunk: 2 PSUM banks, two
                        # 256-col halves; 4 xtg tiles stay live across halves.
                        xtgs = []
                        for g4 in range(4):
                            xtg = xp.tile([128, 4 * 512], F16, tag="xt", name="xt")
                            nc.sync.dma_start(
                                xtg[:].rearrange("p (g i) -> p g i", g=4),
                                xT[g4 * 512 : (g4 + 1) * 512, i0 : i0 + 512].rearrange(
                                    "(g p) i -> p g i", p=128
                                ),
                            )
                            xtgs.append(xtg)
                        for hpair in range(2):
                            km = [psp.tile([128, 512], F32, tag="kps", name="kps", bufs=2) for _ in range(2)]
                            for g4 in range(4):
                                for g in range(4):
                                    c = g4 * 4 + g
                                    st = c == 0
                                    sp = c == NTC - 1
                                    xt_c = xtgs[g4][:, g * 512 : (g + 1) * 512]
                                    for hh in range(2):
                                        h = 2 * hpair + hh
                                        nc.tensor.matmul(
                                            km[hh][:],
                                            wkg[g4][:, g * 512 + h * 128 : g * 512 + (h + 1) * 128],
                                            xt_c,
                                            start=st, stop=sp,
                                        )
                            for hh in range(2):
                                h = 2 * hpair + hh
                                nc.vector.tensor_copy(
                                    kt[h][:, i0 : i0 + 512], km[hh][:]
                                )
                        for hpair in range(2):
                            qm = [psp.tile([128, 512], F32, tag="kps", name="qm", bufs=2) for _ in range(2)]
                            for g4 in range(4):
                                for g in range(4):
                                    c = g4 * 4 + g
                                    st = c == 0
                                    sp = c == NTC - 1
                                    xt_c = xtgs[g4][:, g * 512 : (g + 1) * 512]
                                    for hh in range(2):
                                        h = 2 * hpair + hh
                                        nc.tensor.matmul(
                                            qm[hh][:],
                                            wqg[g4][:, g * 512 + h * 128 : g * 512 + (h + 1) * 128],
                                            xt_c,
                                            start=st, stop=sp,
                                        )
                            for hh in range(2):
                                h = 2 * hpair + hh
                                nc.vector.tensor_copy(
                                    qt[h][:, i0 : i0 + 512], qm[hh][:]
                                )

                        # Attention for this chunk.
                        nj = 4 * (ic + 1)
                        for h in range(HPG):
                            ctxps = psp.tile([128, 512], F32, tag="ctxps", name="ctxps", bufs=1)
                            rsacc = rsp.tile([128, 512], F32R, tag="rsacc", name="rsacc", bufs=3)
                            for p in range(nj // 2):
                                stp = psp.tile([128, 1024], F32, tag="stps", name="stps", bufs=2)
                                et = etp.tile([128, 1024], F16, tag="et", name="et", bufs=6)
                                rels = []
                                for k2 in (0, 1):
                                    jb = 2 * p + k2
                                    j0 = jb * 128
                                    rel = max(i0, j0) - i0
                                    base = 512 * k2
                                    nc.tensor.matmul(
                                        stp[:, base + rel : base + 512],
                                        kt[h][:, j0 : j0 + 128],
                                        qt[h][:, i0 + rel : i0 + 512],
                                        start=True, stop=True,
                                    )
                                    rels.append(rel)
                                # One exp over both tiles; the gap region of
                                # diagonal pairs holds stale-PSUM exp values,
                                # never read downstream.
                                nc.scalar.activation(
                                    et[:, rels[0] : 1024], stp[:, rels[0] : 1024],
                                    EXP, bias=0.0, scale=SCALE,
                                )
                                for k2 in (0, 1):
                                    jb = 2 * p + k2
                                    rel = rels[k2]
                                    base = 512 * k2
                                    if 2 * p + k2 >= 4 * ic:
                                        nc.vector.tensor_mul(
                                            et[:, base + rel : base + rel + 128],
                                            et[:, base + rel : base + rel + 128],
                                            maskm_t[:],
                                        )
                                    if jb == 0:
                                        nc.gpsimd.tensor_copy(rsacc[:], et[:, 0:512])
                                    else:
                                        nc.gpsimd.tensor_add(
                                            rsacc[:, rel:512],
                                            rsacc[:, rel:512],
                                            et[:, base + rel : base + 512],
                                        )
                                    nc.tensor.matmul(
                                        ctxps[:, rel:512],
                                        vg[jb // 4][
                                            :, (jb % 4) * DG + h * 128 : (jb % 4) * DG + (h + 1) * 128
                                        ],
                                        et[:, base + rel : base + 512],
                                        start=(jb == 0), stop=(jb == nj - 1),
                                    )
                            rsps = psp.tile([128, 512], F32, tag="rsps", name="rsps", bufs=1)
                            nc.tensor.matmul(
                                rsps[:], ones_t[:], rsacc[:], start=True, stop=True
                            )
                            rrb = etp.tile([128, 512], F32, tag="rrb", name="rrb")
                            nc.vector.reciprocal(rrb[:], rsps[:])
                            nc.vector.tensor_mul(
                                ctxt[h][:, i0 : i0 + 512], ctxps[:], rrb[:]
                            )

                    # ---- Out-projection (reuses the K-proj PSUM banks) ----
                    for it in range(NIT):
                        t0 = it * 128
                        ysb = etp.tile([128, D], F16, tag="ysb", name="ysb")
                        for oc in range(4):
                            o0 = oc * 512
                            yps = psp.tile([128, 512], F32, tag="kps", name="yps", bufs=2)
                            for h in range(HPG):
                                nc.tensor.matmul(
                                    yps[:],
                                    ctxt[h][:, t0 : t0 + 128],
                                    wo[h][:, o0 : o0 + 512],
                                    start=(h == 0), stop=(h == HPG - 1),
                                )
                            nc.vector.tensor_copy(ysb[:, o0 : o0 + 512], yps[:])
                        nc.sync.dma_start(y[t0 : t0 + 128, :], ysb[:])
    nc.finalize()
    return nc


def get_nc():
    if "nc" not in _CACHE:
        _CACHE["nc"] = _build()
    return _CACHE["nc"]


def make_in_maps(inputs, w_q, w_k, w_v, w_o, b_o):
    x = np.asarray(inputs, dtype=np.float32)
    w_q = np.asarray(w_q, dtype=np.float32)
    w_k = np.asarray(w_k, dtype=np.float32)
    w_v = np.asarray(w_v, dtype=np.float32)
    w_o = np.asarray(w_o, dtype=np.float32)

    ones = np.ones((128, 128), dtype=np.float32)

    xTs = [np.ascontiguousarray(x[b].T.astype(np.float16)) for b in range(B)]
    wqTs = [np.ascontiguousarray(w_q[g * DG : (g + 1) * DG, :].T.astype(np.float16)) for g in range(G)]
    wkTs = [np.ascontiguousarray(w_k[g * DG : (g + 1) * DG, :].T.astype(np.float16)) for g in range(G)]
    wvTs = [np.ascontiguousarray(w_v[g * DG : (g + 1) * DG, :].T.astype(np.float16)) for g in range(G)]
    woTs = [np.ascontiguousarray(w_o[:, g * DG : (g + 1) * DG].T.astype(np.float16)) for g in range(G)]

    in_maps = []
    for core in range(NCORES):
        b, g = divmod(core, G)
        in_maps.append(
            {
                "xT": xTs[b],
                "wqT": wqTs[g],
                "wkT": wkTs[g],
                "wvT": wvTs[g],
                "woT": woTs[g],
                "maskm": np.triu(np.ones((128, 128), dtype=np.float16)),
                "ones": ones,
            }
        )
    return in_maps


def assemble(results, b_o):
    out = np.zeros((B, S, D), dtype=np.float32)
    for core in range(NCORES):
        b = core // G
        out[b] += results[core]["y"].astype(np.float32)
    out += np.asarray(b_o, dtype=np.float32)[None, None, :]
    return out


def kernel(inputs, w_q, w_k, w_v, w_o, b_o):
    nc = get_nc()
    in_maps = make_in_maps(inputs, w_q, w_k, w_v, w_o, b_o)
    res = run_bass_kernel_spmd(nc, in_maps, core_ids=list(range(NCORES)))
    return assemble(res.results, b_o)


# revision 20
# speedup vs baseline: 1.2235x; 1.0514x over previous
"""MultiHeadAttention Trainium2 kernel, 8-core SPMD.

Sharding: core = (batch b, head-group g), b in {0,1}, g in {0..3}.
Each core computes 4 heads of one batch (tensor-parallel on heads,
data-parallel on batch). Out-projection partials (and the bias) are
summed on host.

Structure per core:
  Phase 1a: V projection only (4 PSUM banks, deep double-buffering).
  Interleaved region: K and Q projections (2 shared PSUM banks, one full-width
    accumulation stream per bank, heads in pairs x two contraction
    passes), attention per 512-chunk (scores pairs in 2-bank PSUM
    mega-tiles -> one exp -> DVE mask on diagonal squares -> PV;
    softmax row-sums accumulate on the Pool engine with one
    ones-matmul per (chunk, head)), then out-projection (reusing the
    K-proj PSUM banks). The tile scheduler interleaves these streams so
    the PE always has independent GEMM work while Act/Pool/DVE chase
    the attention chain.

All matmul inputs fp16 (full PE rate at any tile size); accumulation
fp32 in PSUM. Self-contained: hardcodes B=2, S=2048, D=2048, H=16.
"""

import numpy as np

import concourse.bacc as bacc
import concourse.mybir as mybir
import concourse.tile as tile
from concourse.bass_utils import run_bass_kernel_spmd

B, S, D = 2, 2048, 2048
H = 16
HD = D // H          # 128 head dim
G = 4                # head groups (tensor parallel degree)
HPG = H // G         # 4 heads per group
DG = HPG * HD        # 512 features per group
NCORES = 8
NTC = D // 128       # 16 contraction chunks
NIT = S // 128       # 16 seq tiles of 128
NSC = S // 512       # 4 seq chunks of 512
SCALE = float(1.0 / np.sqrt(np.float32(S)))

F32 = mybir.dt.float32
F32R = mybir.dt.float32r
F16 = mybir.dt.float16
EXP = mybir.ActivationFunctionType.Exp

_CACHE = {}


def _build(nreps=1, trace_sim=False):
    nc = bacc.Bacc(target_bir_lowering=False, trn_type="TRN2")
    xT = nc.dram_tensor("xT", [D, S], F16, kind="ExternalInput")
    wqT = nc.dram_tensor("wqT", [D, DG], F16, kind="ExternalInput")
    wkT = nc.dram_tensor("wkT", [D, DG], F16, kind="ExternalInput")
    wvT = nc.dram_tensor("wvT", [D, DG], F16, kind="ExternalInput")
    woT = nc.dram_tensor("woT", [DG, D], F16, kind="ExternalInput")
    maskm = nc.dram_tensor("maskm", [128, 128], F16, kind="ExternalInput")
    ones = nc.dram_tensor("ones", [128, 128], F32R, kind="ExternalInput")
    y = nc.dram_tensor("y", [S, D], F16, kind="ExternalOutput")

    with tile.TileContext(nc, trace_sim=trace_sim) as tc:
      for _rep in range(nreps):
        with tc.tile_pool(name="res", bufs=1) as res:
            qt = [res.tile([128, S], F16, tag=f"qt{h}", name=f"qt{h}") for h in range(HPG)]
            kt = [res.tile([128, S], F16, tag=f"kt{h}", name=f"kt{h}") for h in range(HPG)]
            vg = [res.tile([128, 4 * DG], F16, tag=f"vg{j}", name=f"vg{j}") for j in range(4)]
            ctxt = [res.tile([128, S], F16, tag=f"ctx{h}", name=f"ctx{h}") for h in range(HPG)]
            wo = [res.tile([128, D], F16, tag=f"wo{h}", name=f"wo{h}") for h in range(HPG)]
            rrbs = [res.tile([128, S], F16, tag=f"rrb{h}", name=f"rrb{h}") for h in range(HPG)]
            wkg = [res.tile([128, 4 * DG], F16, tag=f"wkg{g}", name=f"wkg{g}") for g in range(4)]
            maskm_t = res.tile([128, 128], F16, tag="maskm", name="maskm_t")
            ones_t = res.tile([128, 128], F32R, tag="ones", name="ones_t")


            with (
                tc.tile_pool(name="wqv", bufs=1) as wp,
                tc.tile_pool(name="xstream", bufs=8) as xp,
                tc.tile_pool(name="ph2w", bufs=3) as etp,
                tc.tile_pool(name="rsa", bufs=2) as rsp,
            ):
                wqg = [wp.tile([128, 4 * DG], F16, tag=f"wqg{g}", name=f"wqg{g}") for g in range(4)]
                wvg = [wp.tile([128, 4 * DG], F16, tag=f"wvg{g}", name=f"wvg{g}") for g in range(4)]
                # weights: scalar queue, first-needed first
                for g4 in range(4):
                    nc.scalar.dma_start(
                        wqg[g4][:].rearrange("p (g d) -> p g d", g=4),
                        wqT[g4 * 512 : (g4 + 1) * 512, :].rearrange("(g p) d -> p g d", p=128),
                    )
                    nc.scalar.dma_start(
                        wvg[g4][:].rearrange("p (g d) -> p g d", g=4),
                        wvT[g4 * 512 : (g4 + 1) * 512, :].rearrange("(g p) d -> p g d", p=128),
                    )
                for g4 in range(4):
                    nc.scalar.dma_start(
                        wkg[g4][:].rearrange("p (g d) -> p g d", g=4),
                        wkT[g4 * 512 : (g4 + 1) * 512, :].rearrange("(g p) d -> p g d", p=128),
                    )
                nc.scalar.dma_start(maskm_t[:], maskm[:])
                nc.scalar.dma_start(ones_t[:], ones[:])
                for h in range(HPG):
                    nc.scalar.dma_start(wo[h][:], woT[h * 128 : (h + 1) * 128, :])

                # ---- Phase 1a: V projection ----
                with tc.tile_pool(name="ps1", bufs=8, space="PSUM") as pp1:
                    for ic in range(NSC):
                        i0 = ic * 512
                        vps = [pp1.tile([128, DG], F32, tag="projps", name="projps") for _ in range(4)]
                        for g4 in range(4):
                            xtg = xp.tile([128, 4 * 512], F16, tag="xt", name="xt")
                            nc.sync.dma_start(
                                xtg[:].rearrange("p (g i) -> p g i", g=4),
                                xT[g4 * 512 : (g4 + 1) * 512, i0 : i0 + 512].rearrange(
                                    "(g p) i -> p g i", p=128
                                ),
                            )
                            for g in range(4):
                                c = g4 * 4 + g
                                st = c == 0
                                sp = c == NTC - 1
                                for jj in range(4):
                                    nc.tensor.matmul(
                                        vps[jj][:],
                                        xtg[:, g * 512 + jj * 128 : g * 512 + (jj + 1) * 128],
                                        wvg[g4][:, g * 512 : (g + 1) * 512],
                                        start=st, stop=sp,
                                    )
                        for jj in range(4):
                            nc.vector.tensor_copy(
                                vg[ic][:, jj * DG : (jj + 1) * DG], vps[jj][:]
                            )

                # ---- Interleaved region: K proj + attention + out-proj ----
                with tc.tile_pool(name="psI", bufs=2, space="PSUM") as psp:
                    for ic in range(NSC):
                        i0 = ic * 512
                        # K projection for this chunk: 2 PSUM banks, two
                        # 256-col halves; 4 xtg tiles stay live across halves.
                        xtgs = []
                        for g4 in range(4):
                            xtg = xp.tile([128, 4 * 512], F16, tag="xt", name="xt")
                            nc.sync.dma_start(
                                xtg[:].rearrange("p (g i) -> p g i", g=4),
                                xT[g4 * 512 : (g4 + 1) * 512, i0 : i0 + 512].rearrange(
                                    "(g p) i -> p g i", p=128
                                ),
                            )
                            xtgs.append(xtg)
                        for hpair in range(2):
                            km = [psp.tile([128, 512], F32, tag="kps", name="kps", bufs=2) for _ in range(2)]
                            for g4 in range(4):
                                for g in range(4):
                                    c = g4 * 4 + g
                                    st = c == 0
                                    sp = c == NTC - 1
                                    xt_c = xtgs[g4][:, g * 512 : (g + 1) * 512]
                                    for hh in range(2):
                                        h = 2 * hpair + hh
                                        nc.tensor.matmul(
                                            km[hh][:],
                                            wkg[g4][:, g * 512 + h * 128 : g * 512 + (h + 1) * 128],
                                            xt_c,
                                            start=st, stop=sp,
                                        )
                            for hh in range(2):
                                h = 2 * hpair + hh
                                nc.vector.tensor_copy(
                                    kt[h][:, i0 : i0 + 512], km[hh][:]
                                )
                        for hpair in range(2):
                            qm = [psp.tile([128, 512], F32, tag="kps", name="qm", bufs=2) for _ in range(2)]
                            for g4 in range(4):
                                for g in range(4):
                                    c = g4 * 4 + g
                                    st = c == 0
                                    sp = c == NTC - 1
                                    xt_c = xtgs[g4][:, g * 512 : (g + 1) * 512]
                                    for hh in range(2):
                                        h = 2 * hpair + hh
                                        nc.tensor.matmul(
                                            qm[hh][:],
                                            wqg[g4][:, g * 512 + h * 128 : g * 512 + (h + 1) * 128],
                                            xt_c,
                                            start=st, stop=sp,
                                        )
                            for hh in range(2):
                                h = 2 * hpair + hh
                                nc.vector.tensor_copy(
                                    qt[h][:, i0 : i0 + 512], qm[hh][:]
                                )

                        # Attention for this chunk.
                        nj = 4 * (ic + 1)
                        for h in range(HPG):
                            ctxps = psp.tile([128, 512], F32, tag="ctxps", name="ctxps", bufs=1)
                            rsacc = rsp.tile([128, 512], F32R, tag="rsacc", name="rsacc", bufs=3)
                            for p in range(nj // 2):
                                stp = psp.tile([128, 1024], F32, tag="stps", name="stps", bufs=2)
                                et = etp.tile([128, 1024], F16, tag="et", name="et", bufs=5)
                                rels = []
                                for k2 in (0, 1):
                                    jb = 2 * p + k2
                                    j0 = jb * 128
                                    rel = max(i0, j0) - i0
                                    base = 512 * k2
                                    nc.tensor.matmul(
                                        stp[:, base + rel : base + 512],
                                        kt[h][:, j0 : j0 + 128],
                                        qt[h][:, i0 + rel : i0 + 512],
                                        start=True, stop=True,
                                    )
                                    rels.append(rel)
                                # One exp over both tiles; the gap region of
                                # diagonal pairs holds stale-PSUM exp values,
                                # never read downstream.
                                nc.scalar.activation(
                                    et[:, rels[0] : 1024], stp[:, rels[0] : 1024],
                                    EXP, bias=0.0, scale=SCALE,
                                )
                                for k2 in (0, 1):
                                    jb = 2 * p + k2
                                    rel = rels[k2]
                                    base = 512 * k2
                                    if 2 * p + k2 >= 4 * ic:
                                        nc.vector.tensor_mul(
                                            et[:, base + rel : base + rel + 128],
                                            et[:, base + rel : base + rel + 128],
                                            maskm_t[:],
                                        )
                                    if jb == 0:
                                        nc.gpsimd.tensor_copy(rsacc[:], et[:, 0:512])
                                    else:
                                        nc.gpsimd.tensor_add(
                                            rsacc[:, rel:512],
                                            rsacc[:, rel:512],
                                            et[:, base + rel : base + 512],
                                        )
                                    nc.tensor.matmul(
                                        ctxps[:, rel:512],
                                        vg[jb // 4][
                                            :, (jb % 4) * DG + h * 128 : (jb % 4) * DG + (h + 1) * 128
                                        ],
                                        et[:, base + rel : base + 512],
                                        start=(jb == 0), stop=(jb == nj - 1),
                                    )
                            rsps = psp.tile([128, 512], F32, tag="rsps", name="rsps", bufs=1)
                            nc.tensor.matmul(
                                rsps[:], ones_t[:], rsacc[:], start=True, stop=True
                            )
                            nc.vector.tensor_copy(ctxt[h][:, i0 : i0 + 512], ctxps[:])
                            with nc.allow_low_precision(reason="softmax denom reciprocal, f16 ok at 2e-2 gate"):
                                nc.vector.reciprocal(rrbs[h][:, i0 : i0 + 512], rsps[:])

                    # ---- Out-projection (reuses the K-proj PSUM banks) ----
                    for it in range(NIT):
                        t0 = it * 128
                        ysb = etp.tile([128, D], F16, tag="ysb", name="ysb", bufs=2)
                        nctx = [etp.tile([128, 128], F16, tag=f"nctx{h}", name=f"nctx{h}", bufs=2) for h in range(HPG)]
                        for oc in range(4):
                            o0 = oc * 512
                            yps = psp.tile([128, 512], F32, tag="kps", name="yps", bufs=2)
                            for h in range(HPG):
                                if oc == 0:
                                    nc.vector.tensor_mul(
                                        nctx[h][:],
                                        ctxt[h][:, t0 : t0 + 128],
                                        rrbs[h][:, t0 : t0 + 128],
                                    )
                                nc.tensor.matmul(
                                    yps[:],
                                    nctx[h][:],
                                    wo[h][:, o0 : o0 + 512],
                                    start=(h == 0), stop=(h == HPG - 1),
                                )
                            nc.vector.tensor_copy(ysb[:, o0 : o0 + 512], yps[:])
                        nc.sync.dma_start(y[t0 : t0 + 128, :], ysb[:])
    nc.finalize()
    return nc


def get_nc():
    if "nc" not in _CACHE:
        _CACHE["nc"] = _build()
    return _CACHE["nc"]


def make_in_maps(inputs, w_q, w_k, w_v, w_o, b_o):
    x = np.asarray(inputs, dtype=np.float32)
    w_q = np.asarray(w_q, dtype=np.float32)
    w_k = np.asarray(w_k, dtype=np.float32)
    w_v = np.asarray(w_v, dtype=np.float32)
    w_o = np.asarray(w_o, dtype=np.float32)

    ones = np.ones((128, 128), dtype=np.float32)

    xTs = [np.ascontiguousarray(x[b].T.astype(np.float16)) for b in range(B)]
    wqTs = [np.ascontiguousarray(w_q[g * DG : (g + 1) * DG, :].T.astype(np.float16)) for g in range(G)]
    wkTs = [np.ascontiguousarray(w_k[g * DG : (g + 1) * DG, :].T.astype(np.float16)) for g in range(G)]
    wvTs = [np.ascontiguousarray(w_v[g * DG : (g + 1) * DG, :].T.astype(np.float16)) for g in range(G)]
    woTs = [np.ascontiguousarray(w_o[:, g * DG : (g + 1) * DG].T.astype(np.float16)) for g in range(G)]

    in_maps = []
    for core in range(NCORES):
        b, g = divmod(core, G)
        in_maps.append(
            {
                "xT": xTs[b],
                "wqT": wqTs[g],
                "wkT": wkTs[g],
                "wvT": wvTs[g],
                "woT": woTs[g],
                "maskm": np.triu(np.ones((128, 128), dtype=np.float16)),
                "ones": ones,
            }
        )
    return in_maps


def assemble(results, b_o):
    out = np.zeros((B, S, D), dtype=np.float32)
    for core in range(NCORES):
        b = core // G
        out[b] += results[core]["y"].astype(np.float32)
    out += np.asarray(b_o, dtype=np.float32)[None, None, :]
    return out


def kernel(inputs, w_q, w_k, w_v, w_o, b_o):
    nc = get_nc()
    in_maps = make_in_maps(inputs, w_q, w_k, w_v, w_o, b_o)
    res = run_bass_kernel_spmd(nc, in_maps, core_ids=list(range(NCORES)))
    return assemble(res.results, b_o)
